# revision 37
# baseline (speedup 1.0000x reference)
"""Bipolar self-attention on 8 Trainium2 NeuronCores.

Sharding: data-parallel over batch (B=2 -> 2 groups of 4 cores), tensor-
parallel over heads within a group (16 heads -> 4 heads/core). Each core:
  - projects its head-slice of Q/K transposed ([c, n] layout) and V natural,
    with the bipolar transform (q-0.5)*2 and the 1/sqrt(Dh) score scale
    folded into the projection weights/biases host-side,
  - computes S^T = Kb Qb^T per head tile-by-tile, exponentiates (softmax
    without max subtraction -- scores are O(10), exp is safe in fp32),
  - multiplies P^T by a per-(pair, half) [V | ones] stationary block (128
    cols) so PSUM rows 0-63 hold the attention output and rows 64-127 hold
    the softmax denominator already replicated across 64 partitions.
    Matmul cost depends only on the moving dim, so this denominator
    broadcast is free,
  - normalizes with tensor_copy (den -> SBUF; the custom-DVE reciprocal
    misreads PSUM at partition offset 64) + reciprocal_approx_fast +
    tensor_mul (no DRAM broadcast roundtrip, no PSUM evacuation copy),
  - applies its slice of the output projection (row-parallel).
Host sums the 4 partial outputs per batch and adds the bias terms.

Score path (x, Wq/Wk, Q^T/K^T) stays float32r (1 cycle/row at moving>=256;
bf16 there costs ~1e-2 of the 2e-2 error budget for no cycle gain); the
post-softmax path (P, V, out, Wo) is bf16, which cuts SBUF/DMA energy and
measurably reduces DVFS throttling.  The PE executes in order, so
projection / output-projection matmuls are interleaved INTO the attention
k-tile loops (window (0,0) weaves the k-projection chunks paced to the x
block DMAs); emission is pair-major, with y(qq-1) tiles filling the pair-1
windows.  Denser schedules (v7/v9 experiments) stretched ACT exp execution
~20% via shared power/SBUF-port pressure and were net slower.
"""

import ml_dtypes
import numpy as np

import concourse.bass as bass
import concourse.tile as tile
from concourse import bacc, mybir
from concourse.bass_utils import run_bass_kernel_spmd

D_MODEL = 1024
NHEAD = 16
HEAD_DIM = 64
B = 2
N = 2048
N_CORES = 8
HEADS_PER_CORE = NHEAD // (N_CORES // B)  # 4
C_LOC = HEADS_PER_CORE * HEAD_DIM  # 256

F32 = mybir.dt.float32
F32R = mybir.dt.float32r
BF16 = mybir.dt.bfloat16

_CACHE = {}


def build_nc():
    nc = bacc.Bacc("TRN2", target_bir_lowering=False, debug=False)

    xT = nc.dram_tensor("xT", [D_MODEL, N], BF16, kind="ExternalInput")
    wqT = nc.dram_tensor("wqT", [D_MODEL, C_LOC], BF16, kind="ExternalInput")
    wkT = nc.dram_tensor("wkT", [D_MODEL, C_LOC], BF16, kind="ExternalInput")
    wvT = nc.dram_tensor("wvT", [D_MODEL, C_LOC], BF16, kind="ExternalInput")
    woT = nc.dram_tensor("woT", [C_LOC, D_MODEL], BF16, kind="ExternalInput")
    bq = nc.dram_tensor("bq", [C_LOC], F32, kind="ExternalInput")
    bk = nc.dram_tensor("bk", [C_LOC], F32, kind="ExternalInput")
    # block layout [nt, cok, 128, 512] so each output DMA is one fully
    # contiguous 128KB burst (the host re-tiles); bf16 halves the bytes.
    y = nc.dram_tensor("y", [N // 128, D_MODEL // 512, 128, 512], BF16,
                       kind="ExternalOutput")

    NT = N // 128          # 16 k tiles
    DC = D_MODEL // 128    # 8 contraction chunks
    CT = C_LOC // 128      # 2 local-channel tiles (= head pairs)
    QW = 512               # q window width
    NW = N // QW           # 4 q windows

    with tile.TileContext(nc) as tc:
        with (
            tc.tile_pool(name="singles", bufs=1) as singles,
            tc.tile_pool(name="pt", bufs=4) as ptp,
            tc.tile_pool(name="rec", bufs=4) as recp,
            tc.tile_pool(name="yout", bufs=3) as youtp,
        ):
            # Input DMAs ride the SP ring in priority order (DMA bandwidth
            # is ~200-300GB/s shared, so a second queue only lets
            # low-priority transfers steal from the critical prefix).
            # Weights are split by ct so the first projection chain starts
            # on a 0.25MB prefix; x blocks 1-3 are one batched DMA each
            # (their consumers need all dc of a block anyway, and each
            # dma_start costs ~650ns of serialized queue time).
            # Deadlines: blk1 ~23us (v_proj kt4), blk2/3 ~28/33us, wv
            # ~18us (v_proj kt0), ct1 weights ~45us, wo ~120us.
            wqT_sb = singles.tile([128, DC, C_LOC], BF16)
            wkT_sb = singles.tile([128, DC, C_LOC], BF16)
            wvT_sb = singles.tile([128, DC, C_LOC], BF16)
            xT_sb = singles.tile([128, DC, N], BF16)
            bq_sb = singles.tile([128, CT], F32)
            bk_sb = singles.tile([128, CT], F32)
            woT_sb = singles.tile([128, CT, D_MODEL], BF16)
            xT_r = xT.ap().rearrange("(c p) n -> p c n", p=128)
            wq_r = wqT.ap().rearrange("(c p) m -> p c m", p=128)
            wk_r = wkT.ap().rearrange("(c p) m -> p c m", p=128)
            # All input DMAs on the SP ring in priority order (the rings
            # share ~300GB/s of DMA bandwidth, so a second queue only lets
            # low-priority transfers steal from the critical prefix).
            nc.sync.dma_start(wqT_sb[:, :, 0:128], wq_r[:, :, 0:128])
            nc.sync.dma_start(wkT_sb[:, :, 0:128], wk_r[:, :, 0:128])
            nc.sync.dma_start(bq_sb[:], bq.ap().rearrange("(c p) -> p c", p=128))
            nc.sync.dma_start(bk_sb[:], bk.ap().rearrange("(c p) -> p c", p=128))
            for dc in range(DC):
                nc.sync.dma_start(xT_sb[:, dc, 0:QW], xT_r[:, dc, 0:QW])
            nc.sync.dma_start(wvT_sb[:], wvT.ap().rearrange("(c p) m -> p c m", p=128))
            nc.sync.dma_start(wqT_sb[:, :, 128:256], wq_r[:, :, 128:256])
            nc.sync.dma_start(wkT_sb[:, :, 128:256], wk_r[:, :, 128:256])
            for blk in range(1, NW):
                nc.sync.dma_start(
                    xT_sb[:, :, blk * QW:(blk + 1) * QW],
                    xT_r[:, :, blk * QW:(blk + 1) * QW],
                )
            nc.sync.dma_start(woT_sb[:], woT.ap().rearrange("(c p) m -> p c m", p=128))

            qT_sb = singles.tile([128, CT, N], BF16)
            kT_sb = singles.tile([128, CT, N], BF16)
            # V stationary blocks: per (k-tile, pair, half) a [128, 128]
            # block [ones (64) | V_head (64)]: PV output rows 0:64 hold the
            # softmax denominator replicated across partitions (broadcast for
            # free -- matmul cost depends only on the moving dim) and rows
            # 64:128 the attention output.  Den-first so the custom-DVE
            # reciprocal reads PSUM at partition offset 0 (it misreads PSUM
            # at offset 64), killing the den-bounce tensor_copy.
            v1_sb = singles.tile([128, NT, CT, 2, 128], BF16)
            ones_sb = singles.tile([128, 128], F32)
            nc.vector.memset(ones_sb[:], 1.0)
            # bf16 junk tile for PE warm-up matmuls (no DMA dependency)
            warm_sb = singles.tile([128, 512], BF16)
            nc.vector.memset(warm_sb[:], 0.0)
            for nt in range(NT):
                for pair in range(CT):
                    nc.vector.tensor_copy(
                        v1_sb[:, nt, pair, :, 0:64],
                        ones_sb[:].rearrange("p (h d) -> p h d", h=2),
                    )
            outT_sb = singles.tile([128, CT, N], BF16)

            # ---- emission helpers.  All PE work is emitted via closures so
            # the interleaving below is explicit.
            with (
                tc.tile_pool(name="ps512", bufs=2, space="PSUM") as psp,
                tc.tile_pool(name="st_ps", bufs=2, space="PSUM") as stp,
                tc.tile_pool(name="ov_ps", bufs=2, space="PSUM") as ovp,
            ):
                def warmup(n, wide=True):
                    # dummy matmuls on junk data: keep the HAM clock warm
                    # while the PE would otherwise idle (startup DMA wait,
                    # tail normalize chain).  Allocates a ps slot properly so
                    # the pool's WAR tracking orders it vs real users.
                    wp = psp.tile([128, 512], F32, tag="ps", name="warmps")
                    w = 512 if wide else 128
                    for _ in range(n):
                        nc.tensor.matmul(
                            wp[:, 0:w],
                            warm_sb[:, 0:128],
                            warm_sb[:, 0:w],
                            start=True,
                            stop=True,
                        )

                def qk_proj_chunk(w_sb, b_sb, dst, ct, nch):
                    # one 512-wide chunk: 8 matmuls + bias add
                    ps = psp.tile([128, 512], F32, tag="ps")
                    for dc in range(DC):
                        nc.tensor.matmul(
                            ps[:],
                            w_sb[:, dc, ct * 128:(ct + 1) * 128],
                            xT_sb[:, dc, nch * 512:(nch + 1) * 512],
                            start=(dc == 0),
                            stop=(dc == DC - 1),
                        )
                    nc.vector.tensor_tensor(
                        dst[:, ct, nch * 512:(nch + 1) * 512],
                        ps[:],
                        b_sb[:, ct:ct + 1].to_broadcast((128, 512)),
                        mybir.AluOpType.add,
                    )

                def v_proj_tile(nt):
                    # V natural: v[n, c] = sum_d xT[d, n] wvT[d, c], one n tile
                    ps = psp.tile([128, 512], F32, tag="ps")
                    for dc in range(DC):
                        nc.tensor.matmul(
                            ps[:, :C_LOC],
                            xT_sb[:, dc, nt * 128:(nt + 1) * 128],
                            wvT_sb[:, dc, :],
                            start=(dc == 0),
                            stop=(dc == DC - 1),
                        )
                    # scatter the 4 heads into their [pair, half] slots
                    # (V occupies columns 64:128; 0:64 hold the ones block)
                    nc.vector.tensor_copy(
                        v1_sb[:, nt].rearrange("p c h w -> p (c h) w")[:, :, 64:128],
                        ps[:, :C_LOC].rearrange("p (g d) -> p g d", g=4),
                    )

                def y_proj_tile(nt, tail=False):
                    # output projection for one 128-row tile.  In the tail
                    # (after the last exp) the ACT engine is free: run the
                    # two coks' evac+DMA on SEPARATE engine chains (DVE+SP
                    # and ACT+ACT-queue) so they drain in parallel with the
                    # final normalize work.
                    for cok in range(D_MODEL // 512):
                        if tail and cok % 2:
                            # the attention st banks are free in the tail:
                            # use them as extra slots so the y matmuls don't
                            # serialize behind the evacuation copies
                            ps = stp.tile([128, 2 * QW], F32, tag="st",
                                          name="yps_t")[:, 0:512]
                        else:
                            ps = psp.tile([128, 512], F32, tag="ps", name="yps")
                        for ct in range(CT):
                            nc.tensor.matmul(
                                ps[:],
                                outT_sb[:, ct, nt * 128:(nt + 1) * 128],
                                woT_sb[:, ct, cok * 512:(cok + 1) * 512],
                                start=(ct == 0),
                                stop=(ct == CT - 1),
                            )
                        ys = youtp.tile([128, 512], BF16, tag="ys")
                        if tail and cok % 2:
                            nc.scalar.activation(
                                ys[:], ps[:], mybir.ActivationFunctionType.Copy
                            )
                            dma_eng = nc.scalar
                        else:
                            nc.vector.tensor_copy(ys[:], ps[:])
                            dma_eng = nc.sync
                        dma_eng.dma_start(y.ap()[nt, cok], ys[:])

                def attention_window(qq, pair, filler=None):
                    """One q-window of one head pair.  `filler()` is called
                    once per k-tile to emit interleaved PE work."""
                    q0 = qq * QW
                    ovA = ovp.tile([128, QW], F32, tag="ov")
                    ovB = ovp.tile([128, QW], F32, tag="ov")
                    for kt in range(NT):
                        st = stp.tile([128, 2 * QW], F32, tag="st")
                        for half, p0 in ((0, 0), (1, 64)):
                            nc.tensor.matmul(
                                st[:, half * QW:(half + 1) * QW],
                                kT_sb[p0:p0 + 64, pair,
                                      kt * 128:(kt + 1) * 128],
                                qT_sb[p0:p0 + 64, pair, q0:q0 + QW],
                                start=True,
                                stop=True,
                            )
                        pt = ptp.tile([128, 2 * QW], BF16)
                        nc.scalar.activation(
                            pt[:], st[:], mybir.ActivationFunctionType.Exp
                        )
                        if filler is not None:
                            filler(kt)
                        # [ones | V] -> den rows 0:64, out rows 64:128
                        for half, ov in ((0, ovA), (1, ovB)):
                            nc.tensor.matmul(
                                ov[:],
                                v1_sb[:, kt, pair, half],
                                pt[:, half * QW:(half + 1) * QW],
                                start=(kt == 0),
                                stop=(kt == NT - 1),
                            )
                    # normalize: rec = 1/den (broadcast across partitions is
                    # already materialized in PSUM rows 0:64, where the
                    # custom-DVE reciprocal can read PSUM directly), then
                    # outT = out * rec.
                    for half, ov in ((0, ovA), (1, ovB)):
                        p0 = 64 * half
                        rec = recp.tile([64, QW], F32, tag="rec")
                        nc.vector.reciprocal_approx_fast(rec[:], ov[0:64, :])
                        nc.vector.tensor_mul(
                            outT_sb[p0:p0 + 64, pair, q0:q0 + QW],
                            ov[64:128, :],
                            rec[:],
                        )

                # ---- schedule -------------------------------------------
                # Short PE warm-up bridges until the first weight/x slices
                # land (~8.5us); the first projection chain is DMA-paced
                # (one x slice every ~630ns), so q and k matmuls are PAIRED
                # per dc -- each arriving slice feeds both -- with a warm
                # matmul in each remaining gap to keep the HAM ramping.
                # The k projection thereby leaves the critical path.
                warmup(16, wide=False)
                ps_q = psp.tile([128, 512], F32, tag="ps", name="ps_q")
                ps_k = psp.tile([128, 512], F32, tag="ps", name="ps_k")
                wgap = stp.tile([128, 2 * QW], F32, tag="st", name="wgap")
                for dc in range(DC):
                    for w_sb, ps in ((wqT_sb, ps_q), (wkT_sb, ps_k)):
                        nc.tensor.matmul(
                            ps[:],
                            w_sb[:, dc, 0:128],
                            xT_sb[:, dc, 0:512],
                            start=(dc == 0),
                            stop=(dc == DC - 1),
                        )
                    if dc < DC - 1:
                        nc.tensor.matmul(
                            wgap[:, 0:128],
                            warm_sb[:, 0:128],
                            warm_sb[:, 0:128],
                            start=True,
                            stop=True,
                        )
                nc.vector.tensor_tensor(
                    qT_sb[:, 0, 0:512], ps_q[:],
                    bq_sb[:, 0:1].to_broadcast((128, 512)), mybir.AluOpType.add)
                nc.vector.tensor_tensor(
                    kT_sb[:, 0, 0:512], ps_k[:],
                    bk_sb[:, 0:1].to_broadcast((128, 512)), mybir.AluOpType.add)
                # Window (0,0) starts right after the q/k projections of x
                # block 0; its filler emits the later k-projection chunks
                # (paced to the x block DMAs), the V tiles, and at the end
                # the q chunks for windows 1-3.

                def fill_w00(kt):
                    # v tile exactly when its PV needs it (emitted right
                    # after this filler), so the first exp isn't pushed out
                    # by cold-p-state V projections.
                    v_proj_tile(kt)
                    if kt % 4 == 2 and kt // 4 + 1 < NW:
                        qk_proj_chunk(wkT_sb, bk_sb, kT_sb, 0, kt // 4 + 1)
                    if kt == 14:
                        # only nch1 is needed by window (1,0); nch2/3 moved
                        # into the proj1 rotation to unclog window 0 (whose
                        # PE load otherwise stalls the exp pipeline ~3us/kt)
                        qk_proj_chunk(wqT_sb, bq_sb, qT_sb, 0, 1)

                attention_window(0, 0, filler=fill_w00)

                # pair-0 windows 1..3; spread the remaining q-ct0 chunks and
                # the pair-1 Q/K projection across their slack with a unit
                # at kt=0 in EVERY window (covers the ov-bank normalize
                # latency at window boundaries).  Deadlines: q-ct0 nch2/3
                # before windows (2,0)/(3,0); k-ct1 before (0,1); q-ct1
                # nch0-2 before (0,1)/(1,1)/(2,1).  q-ct1 nch3 is reserved
                # as window (0,1)'s own kt=0 unit -- it is only read by
                # window (3,1), much later.
                proj1 = [(wqT_sb, bq_sb, qT_sb, 0, 2),
                         (wqT_sb, bq_sb, qT_sb, 0, 3)]
                proj1 += [(wkT_sb, bk_sb, kT_sb, 1, nch) for nch in range(NW)]
                proj1 += [(wqT_sb, bq_sb, qT_sb, 1, nch) for nch in range(NW - 1)]

                def make_fill_proj(chunks, period):
                    it = iter(chunks)

                    def fill(kt):
                        if kt % period == 0:
                            args = next(it, None)
                            if args is not None:
                                w, b, dst, ct, nch = args
                                qk_proj_chunk(w, b, dst, ct, nch)
                    return fill

                fp = make_fill_proj(proj1, 6)
                attention_window(1, 0, filler=fp)
                attention_window(2, 0, filler=fp)
                attention_window(3, 0, filler=fp)

                # pair-1 windows with trailing y projection interleaved.
                # y window qq is ready once pair-1 window qq is normalized.
                def make_fill_y(qq_ready):
                    chunks = list(range(qq_ready * (QW // 128),
                                        (qq_ready + 1) * (QW // 128)))
                    it = iter(chunks)

                    def fill(kt):
                        if kt % 4 == 0:
                            args = next(it, None)
                            if args is not None:
                                y_proj_tile(args)
                    return fill

                fq3 = make_fill_proj(
                    [(wqT_sb, bq_sb, qT_sb, 1, NW - 1)], 16)
                attention_window(0, 1, filler=fq3)
                attention_window(1, 1, filler=make_fill_y(0))
                attention_window(2, 1, filler=make_fill_y(1))
                attention_window(3, 1, filler=make_fill_y(2))
                # bridge the last normalize chain so the HAM stays warm for
                # the trailing y tiles
                warmup(8)
                for nt in range(3 * (QW // 128), NW * (QW // 128)):
                    y_proj_tile(nt, tail=True)

    nc.compile()
    return nc


def kernel(x, Wq, bq, Wk, bk, Wv, bv, Wo, bo):
    x = np.asarray(x, dtype=np.float32)
    Wq = np.asarray(Wq, dtype=np.float32)
    Wk = np.asarray(Wk, dtype=np.float32)
    Wv = np.asarray(Wv, dtype=np.float32)
    Wo = np.asarray(Wo, dtype=np.float32)
    bq = np.asarray(bq, dtype=np.float32)
    bk = np.asarray(bk, dtype=np.float32)
    bv = np.asarray(bv, dtype=np.float32)
    bo = np.asarray(bo, dtype=np.float32)

    if "nc" not in _CACHE:
        _CACHE["nc"] = build_nc()
    nc = _CACHE["nc"]

    s = 2.0 / np.sqrt(8.0)  # fold bipolar *2 and score scale (1/8 split per side)
    in_maps = []
    for core in range(N_CORES):
        b = core // (N_CORES // B)
        g = core % (N_CORES // B)
        ch = slice(g * C_LOC, (g + 1) * C_LOC)
        in_maps.append({
            "xT": np.ascontiguousarray(x[b].T).astype(ml_dtypes.bfloat16),
            "wqT": np.ascontiguousarray((s * Wq[ch, :]).T).astype(ml_dtypes.bfloat16),
            "wkT": np.ascontiguousarray((s * Wk[ch, :]).T).astype(ml_dtypes.bfloat16),
            "wvT": np.ascontiguousarray(Wv[ch, :].T).astype(ml_dtypes.bfloat16),
            "woT": np.ascontiguousarray(Wo[:, ch].T).astype(ml_dtypes.bfloat16),
            "bq": ((2.0 * bq[ch] - 1.0) / np.sqrt(8.0)).astype(np.float32),
            "bk": ((2.0 * bk[ch] - 1.0) / np.sqrt(8.0)).astype(np.float32),
        })

    _CACHE["in_maps"] = in_maps
    res = run_bass_kernel_spmd(nc, in_maps, core_ids=list(range(N_CORES)))

    g_per_b = N_CORES // B
    const = (Wo @ bv + bo).astype(np.float32)  # bv folded through out-proj
    out = np.empty((B, N, D_MODEL), dtype=np.float32)
    for b in range(B):
        acc = None
        for g in range(g_per_b):
            # y block layout [nt, cok, 128, 512] -> [N, D_MODEL]
            yb = res.results[b * g_per_b + g]["y"].astype(np.float32)
            yb = yb.transpose(0, 2, 1, 3).reshape(N, D_MODEL)
            acc = yb if acc is None else acc + yb
        out[b] = acc + const
    return out

